# revision 30
# baseline (speedup 1.0000x reference)
"""Trainium2 Bass kernel for nn_Attention_2 (B=32, LQ=LK=2048, H=1024, A=512).

Math: the reference's softmax is over sum_q(Qp @ Kp^T); the q-sum
distributes through the matmuls so the [B, LQ, LK] score tensor never
exists:

  qs[b]   = sum_q query[b,q,:]                      [H]
  qp[b]   = qs[b] @ Wq + LQ*bq                      [A]
  t[b]    = qp[b] @ Wk^T                            [H]
  s[b,k]  = key[b,k,:] . t[b]     (+ const, cancels in softmax)
  e[b,k]  = exp(s[b,k] - max_k s)
  x[b]    = (sum_k e[b,k] * key[b,k,:]) . u / sum_k e[b,k] + cv
            where u = Wk @ Wv[:,0],  cv = bk.Wv + bv

The last line folds the per-key value v_k = key.u + cv through the
softmax combine, so no per-key v dot products are needed: the weighted
key sum is a post-softmax PE matmul (bf16 is plenty: only the relative
weight of the top few keys matters, tolerance is 2e-2).

Sharding: data-parallel over batch, 4 batches per core, 8 cores.
Per core the only heavy work is streaming query+key (64MB) from HBM as
plain 2MB granule DMAs (query on the scalar queue, key on the sync
queue — accumulate-DMAs run ~1.6x slower per engine, so query is
reduced on-chip instead):
  - query granules: gpsimd elementwise adds into acc[b], then 8 tiny
    PE ones-matmuls fold acc -> qsT columns
  - key subtiles: fp32 DVE fused mul+reduce vs broadcast t[b] (scores
    must be fp32: top-2 logit gaps ~1-4 at |s|~1000), plus a scalar
    engine bf16 cast of each subtile held for the post-softmax kw
    matmul on the PE.
"""
import numpy as np

import concourse.bass as bass
import concourse.bacc as bacc
import concourse.tile as tile
from concourse import mybir
from concourse import bass_isa
from concourse.bass_utils import run_bass_kernel_spmd

N_CORES = 8
B, LQ, LK, H, A = 32, 2048, 2048, 1024, 512
BPC = B // N_CORES          # batches per core
P = 128
f32 = mybir.dt.float32
bf16 = mybir.dt.bfloat16
NG = 4                      # 2MB key granules per batch
GK = LK // NG               # 512 rows per granule
NSUB = GK // P              # 4 subtiles per granule
NGQ = 8                     # 1MB query granules per batch
GQ = LQ // NGQ              # 256 rows per query granule
QSUB = GQ // P              # 2 subtiles per query granule
NKW = 2                     # batches using the PE kw route (rest: DVE v-dots)
NKT = LK // P               # 16 subtiles per batch
HJ = H // P                 # 8 h-chunks
AC = A // P                 # 4 a-chunks

_CACHE = {}
import os as _os
KG_BUFS = int(_os.environ.get("KG_BUFS", "3"))
QG_BUFS = int(_os.environ.get("QG_BUFS", "2"))


def build_bass(repeat=1, variant="full"):
    nc = bacc.Bacc(None, target_bir_lowering=False, debug=False)

    query = nc.dram_tensor("query", [BPC, LQ, H], f32, kind="ExternalInput").ap()
    key = nc.dram_tensor("key", [BPC, LK, H], f32, kind="ExternalInput").ap()
    Wq = nc.dram_tensor("Wq", [H, A], f32, kind="ExternalInput").ap()
    bq = nc.dram_tensor("bq", [A], f32, kind="ExternalInput").ap()
    Wk = nc.dram_tensor("Wk", [H, A], f32, kind="ExternalInput").ap()
    bk = nc.dram_tensor("bk", [A], f32, kind="ExternalInput").ap()
    Wv = nc.dram_tensor("Wv", [A, 1], f32, kind="ExternalInput").ap()
    bv = nc.dram_tensor("bv", [1], f32, kind="ExternalInput").ap()
    out = nc.dram_tensor("out", [BPC, 1], f32, kind="ExternalOutput").ap()

    with tile.TileContext(nc) as tc:
        for _ in range(repeat):
            _build_body(nc, tc, query, key, Wq, bq, Wk, bk, Wv, bv, out,
                        variant=variant)
    nc.compile()
    return nc


def _build_body(nc, tc, query, key, Wq, bq, Wk, bk, Wv, bv, out, variant="full"):
    from contextlib import ExitStack
    ctx = ExitStack()
    with ctx:
        sbc = ctx.enter_context(tc.tile_pool(name="sbc", bufs=1))
        sbqa = ctx.enter_context(tc.tile_pool(name="sbqa", bufs=2))
        sbq = ctx.enter_context(tc.tile_pool(name="sbq", bufs=QG_BUFS))
        sbkey = ctx.enter_context(tc.tile_pool(name="sbkey", bufs=KG_BUFS))
        sbkbf = ctx.enter_context(tc.tile_pool(name="sbkbf", bufs=3))
        sbw = ctx.enter_context(tc.tile_pool(name="sbw", bufs=2))
        sbsm = ctx.enter_context(tc.tile_pool(name="sbsm", bufs=2))
        sbjunk = ctx.enter_context(tc.tile_pool(name="sbjunk", bufs=1, space="PSUM"))
        ps_small = ctx.enter_context(tc.tile_pool(name="ps_small", bufs=2, space="PSUM"))
        ps_kw = ctx.enter_context(tc.tile_pool(name="ps_kw", bufs=4, space="PSUM"))

        if variant == "dma":
            # pure-DMA roofline probe: all loads, no compute
            for b in range(BPC):
                for g in range(NGQ):
                    qt = sbq.tile([P, QSUB * H], f32, tag="qg")
                    nc.scalar.dma_start(
                        out=qt[:].rearrange("p (n h) -> p n h", n=QSUB),
                        in_=query[b, g * GQ:(g + 1) * GQ, :]
                        .rearrange("(n p) h -> p n h", p=P))
                for g in range(NG):
                    kt = sbkey.tile([P, NSUB * H], f32, tag="kg")
                    nc.sync.dma_start(
                        out=kt[:].rearrange("p (n h) -> p n h", n=NSUB),
                        in_=key[b, g * GK:(g + 1) * GK, :]
                        .rearrange("(n p) h -> p n h", p=P))
            xz = sbsm.tile([1, 1], f32, tag="x")
            nc.vector.memset(xz[:], 0.0)
            for b in range(BPC):
                nc.sync.dma_start(out=out[b:b + 1, :], in_=xz[:])
            return

        # ---------------- constants ----------------
        ident = sbc.tile([P, P], f32)
        colidx = sbsm.tile([P, P], f32, tag="small", bufs=1)
        rowidx = sbsm.tile([P, 1], f32, tag="tiny")
        nc.gpsimd.iota(colidx[:], pattern=[[1, P]], base=0, channel_multiplier=0,
                       allow_small_or_imprecise_dtypes=True)
        nc.gpsimd.iota(rowidx[:], pattern=[[0, 1]], base=0, channel_multiplier=1,
                       allow_small_or_imprecise_dtypes=True)
        nc.vector.tensor_scalar(out=ident[:], in0=colidx[:], scalar1=rowidx[:],
                                scalar2=None, op0=mybir.AluOpType.is_equal)

        ones = sbc.tile([P, 1], f32)            # column of 1s
        nc.vector.memset(ones[:], 1.0)
        ones_k1 = sbc.tile([1, P], f32)         # single-partition row of 1s
        nc.vector.memset(ones_k1[:], 1.0)
        inv128 = sbc.tile([P, 1], f32)          # column of 1/128
        nc.vector.memset(inv128[:], 1.0 / P)
        lqconst = sbc.tile([1, 1], f32)         # LQ (for the bq bias matmul)
        nc.vector.memset(lqconst[:], float(LQ))

        # ---------------- per-batch pieces ----------------
        qacc = [None] * BPC         # pinned first query granule = accumulator
        qacc2 = [None] * BPC        # second accumulation target (g2+g3)
        wqbcs = [None] * BPC
        state = {}
        wvbc_box = [None]

        def is_kw(b):
            return b < NKW

        def emit_query_granule(b, g):
            """2MB query granule, plain load (scalar/gpsimd queues).
            Granule 0 is pinned as the accumulator; 1-3 added by DVE."""
            if g == 0:
                qt = sbqa.tile([P, QSUB * H], f32, tag="qa", name=f"qa{b}")
                qacc[b] = qt
            else:
                qt = sbq.tile([P, QSUB * H], f32, tag="qg", name=f"q{b}_{g}")
            qeng = [nc.scalar, nc.gpsimd, nc.sync][g % 3]
            qeng.dma_start(
                out=qt[:].rearrange("p (n h) -> p n h", n=QSUB),
                in_=query[b, g * GQ:(g + 1) * GQ, :]
                .rearrange("(n p) h -> p n h", p=P))
            if g > 0:
                acc = qacc[b]
                nc.vector.tensor_tensor(out=acc[:], in0=acc[:], in1=qt[:],
                                        op=mybir.AluOpType.add)
                if g == NGQ - 1:
                    # fold the subtile blocks down to acc[:, 0:H]
                    if QSUB == 4:
                        nc.vector.tensor_tensor(out=acc[:, 0:2 * H],
                                                in0=acc[:, 0:2 * H],
                                                in1=acc[:, 2 * H:4 * H],
                                                op=mybir.AluOpType.add)
                    nc.vector.tensor_tensor(out=acc[:, 0:H], in0=acc[:, 0:H],
                                            in1=acc[:, H:2 * H],
                                            op=mybir.AluOpType.add)

        def emit_query(b):
            for g in range(NGQ):
                emit_query_granule(b, g)

        def emit_prep(b):
            """qsT -> qp -> t -> broadcast t across partitions."""
            acc = qacc[b]
            # qsT[m, j] = sum over the 4 subtile blocks and partitions
            qsT_ps = ps_small.tile([P, HJ], f32, tag="small")
            for j in range(HJ):
                nc.tensor.matmul(qsT_ps[:, j:j + 1],
                                 acc[:, j * P:(j + 1) * P],
                                 ones[:], start=True, stop=True)
            qsT_sb = sbsm.tile([P, HJ], f32, tag="qsT", bufs=1)
            nc.scalar.copy(qsT_sb[:], qsT_ps[:])

            # qp = qs @ Wq  [1, A]
            qp_ps = ps_small.tile([1, A], f32, tag="small")
            for j in range(HJ):
                nc.tensor.matmul(qp_ps[:], qsT_sb[:, j:j + 1],
                                 Wq_sb[:, j * A:(j + 1) * A],
                                 start=(j == 0), stop=False)
            nc.tensor.matmul(qp_ps[:], lqconst[:], bq_row[:],
                             start=False, stop=True)
            qp2 = sbsm.tile([1, A], f32, tag="qp2", bufs=1)
            nc.scalar.copy(qp2[:], qp_ps[:])
            qp2bc = sbsm.tile([P, A], f32, tag="qp2bc", bufs=1)
            nc.gpsimd.partition_broadcast(qp2bc[:], qp2[:], channels=P)
            qpT_ps = ps_small.tile([P, AC], f32, tag="small")
            for c in range(AC):
                nc.tensor.matmul(qpT_ps[:, c:c + 1],
                                 qp2bc[:, c * P:(c + 1) * P], inv128[:],
                                 start=True, stop=True)
            qpT_sb = sbsm.tile([P, AC], f32, tag="qpT", bufs=1)
            nc.scalar.copy(qpT_sb[:], qpT_ps[:])

            # t = qp2 @ Wk^T  [1, H] in two halves
            t_sb = sbsm.tile([1, H], f32, tag="t", bufs=1)
            for half in range(2):
                t_ps = ps_small.tile([1, A], f32, tag="small")
                for c in range(AC):
                    nc.tensor.matmul(
                        t_ps[:], qpT_sb[:, c:c + 1],
                        WkT_sb[:, c * H + half * A: c * H + (half + 1) * A],
                        start=(c == 0), stop=(c == AC - 1))
                nc.scalar.copy(t_sb[:, half * A:(half + 1) * A], t_ps[:])

            wqbc = sbw.tile([P, H], f32, tag="wqbc", name=f"wqbc{b}")
            nc.gpsimd.partition_broadcast(wqbc[:], t_sb[:], channels=P)
            wqbcs[b] = wqbc

        def emit_key_granule(b, g):
            """One 2MB key granule: fp32 s-dots; bf16 casts (kw batches) or
            fp32 v-dots (classic batches)."""
            st = state.setdefault(b, {})
            if g == 0:
                st["sdve"] = sbsm.tile([P, NKT], f32, tag="sdve",
                                       name=f"sdve{b}")
                if not is_kw(b):
                    st["vdve"] = sbsm.tile([P, NKT], f32, tag="vdve",
                                           name=f"vdve{b}")
            if is_kw(b) and g % 2 == 0:
                st.setdefault("kbf", []).append(
                    sbkbf.tile([P, (NKT // 2) * H], bf16, tag="kbf",
                               name=f"kbf{b}_{g // 2}"))
            sdve = st["sdve"]
            wqbc = wqbcs[b]
            kt = sbkey.tile([P, NSUB * H], f32, tag="kg", name=f"k{b}_{g}")
            keng = nc.sync if g % 2 == 0 else nc.gpsimd
            keng.dma_start(
                out=kt[:].rearrange("p (n h) -> p n h", n=NSUB),
                in_=key[b, g * GK:(g + 1) * GK, :]
                .rearrange("(n p) h -> p n h", p=P))
            for n in range(NSUB):
                t = g * NSUB + n
                kv = kt[:, n * H:(n + 1) * H]
                junk = sbjunk.tile([P, H], f32, tag="junk")
                nc.vector.scalar_tensor_tensor(
                    out=junk[:], in0=kv, scalar=1.0, in1=wqbc[:],
                    op0=mybir.AluOpType.mult, op1=mybir.AluOpType.mult,
                    accum_out=sdve[:, t:t + 1])
                if is_kw(b):
                    kbf = st["kbf"][g // 2]
                    toff = t - (g // 2) * (NKT // 2)
                    nc.scalar.copy(kbf[:, toff * H:(toff + 1) * H], kv)
                else:
                    junk2 = sbjunk.tile([P, H], f32, tag="junk")
                    nc.vector.scalar_tensor_tensor(
                        out=junk2[:], in0=kv, scalar=1.0, in1=wvbc_box[0][:],
                        op0=mybir.AluOpType.mult, op1=mybir.AluOpType.mult,
                        accum_out=st["vdve"][:, t:t + 1])

        def emit_softmax(b):
            st = state[b]
            sdve = st["sdve"]
            m1 = sbsm.tile([P, 1], f32, tag="m1", bufs=1)
            nc.vector.reduce_max(m1[:], sdve[:], axis=mybir.AxisListType.X)
            mT_ps = ps_small.tile([1, P], f32, tag="small")
            nc.tensor.transpose(mT_ps[:], m1[:], ident[:])
            mT_sb = sbsm.tile([1, P], f32, tag="mT", bufs=1)
            nc.vector.tensor_copy(mT_sb[:], mT_ps[:])
            gmax = sbsm.tile([1, 1], f32, tag="gmax")
            nc.vector.reduce_max(gmax[:], mT_sb[:], axis=mybir.AxisListType.X)
            ng_ps = ps_small.tile([P, 1], f32, tag="small")
            nc.tensor.matmul(ng_ps[:], ones_k1[:], gmax[:], start=True, stop=True)
            ngm = sbsm.tile([P, 1], f32, tag="ngm")
            nc.vector.tensor_scalar_mul(ngm[:], ng_ps[:], -1.0)
            e128 = sbsm.tile([P, NKT], f32, tag="e128", name=f"e128_{b}")
            erow = sbsm.tile([P, 1], f32, tag="erow", name=f"erow{b}")
            nc.scalar.activation(e128[:], sdve[:],
                                 mybir.ActivationFunctionType.Exp,
                                 bias=ngm[:], scale=1.0, accum_out=erow[:])
            st["e128"] = e128
            st["erow"] = erow

        def emit_kw(b):
            st = state[b]
            e_bf = sbsm.tile([P, NKT], bf16, tag="ebf", name=f"ebf{b}")
            nc.scalar.copy(e_bf[:], st["e128"][:])
            kw_ps = [ps_kw.tile([1, A], f32, tag="kw", name=f"kw{b}_{h}")
                     for h in range(2)]
            for t in range(NKT):
                kbf = st["kbf"][t // (NKT // 2)]
                toff = t % (NKT // 2)
                for half in range(2):
                    nc.tensor.matmul(
                        kw_ps[half][:], e_bf[:, t:t + 1],
                        kbf[:, toff * H + half * A: toff * H + (half + 1) * A],
                        start=(t == 0), stop=(t == NKT - 1))
            den_ps = ps_small.tile([1, 1], f32, tag="small")
            nc.tensor.matmul(den_ps[:], st["erow"][:], ones[:],
                             start=True, stop=True)
            den_sb = sbsm.tile([1, 1], f32, tag="den", name=f"den{b}")
            nc.vector.tensor_copy(den_sb[:], den_ps[:])
            st["kw_ps"] = kw_ps
            st["den_sb"] = den_sb

        def emit_fin(b):
            """Final combine for a kw batch; reads kw straight from PSUM."""
            st = state[b]
            kw_ps = st["kw_ps"]
            junk3 = sbjunk.tile([P, H], f32, tag="junk")
            nums = sbsm.tile([1, 2], f32, tag="nums")
            for half in range(2):
                nc.vector.scalar_tensor_tensor(
                    out=junk3[0:1, half * A:(half + 1) * A],
                    in0=kw_ps[half][:], scalar=1.0,
                    in1=u_sb[:, half * A:(half + 1) * A],
                    op0=mybir.AluOpType.mult, op1=mybir.AluOpType.mult,
                    accum_out=nums[:, half:half + 1])
            num = sbsm.tile([1, 1], f32, tag="num")
            nc.vector.tensor_tensor(out=num[:], in0=nums[:, 0:1],
                                    in1=nums[:, 1:2], op=mybir.AluOpType.add)
            _emit_final(num[:], st["den_sb"][:], b)

        def emit_fin3(b):
            """Combine for a classic (v-dot) batch."""
            st = state[b]
            junk5 = sbsm.tile([P, NKT], f32, tag="junk5", bufs=1)
            nrow = sbsm.tile([P, 1], f32, tag="nrow")
            nc.vector.scalar_tensor_tensor(
                out=junk5[:], in0=st["e128"][:], scalar=1.0, in1=st["vdve"][:],
                op0=mybir.AluOpType.mult, op1=mybir.AluOpType.mult,
                accum_out=nrow[:])
            dn_ps = ps_small.tile([1, 2], f32, tag="small")
            nc.tensor.matmul(dn_ps[:, 0:1], st["erow"][:], ones[:],
                             start=True, stop=True)
            nc.tensor.matmul(dn_ps[:, 1:2], nrow[:], ones[:],
                             start=True, stop=True)
            dn = sbsm.tile([1, 2], f32, tag="dn")
            nc.vector.tensor_copy(dn[:], dn_ps[:])
            _emit_final(dn[:, 1:2], dn[:, 0:1], b)

        def _emit_final(num, den, b):
            rden = sbsm.tile([1, 1], f32, tag="rden")
            nc.vector.reciprocal(rden[:], den)
            x = sbsm.tile([1, 1], f32, tag="x")
            nc.vector.tensor_tensor(out=x[:], in0=num, in1=rden[:],
                                    op=mybir.AluOpType.mult)
            x2 = sbsm.tile([1, 1], f32, tag="x2")
            nc.vector.tensor_tensor(out=x2[:], in0=x[:], in1=cv_sb[:],
                                    op=mybir.AluOpType.add)
            nc.sync.dma_start(out=out[b:b + 1, :], in_=x2[:])

        # ---------------- pipeline ----------------
        emit_query(0)

        # Wq natural layout: [h-part, (j a)] ; chunk j at cols [j*A, (j+1)*A)
        Wq_sb = sbc.tile([P, HJ * A], f32)
        nc.sync.dma_start(out=Wq_sb[:].rearrange("p (j a) -> p j a", j=HJ),
                          in_=Wq.rearrange("(j p) a -> p j a", p=P))

        wv_sb = sbc.tile([P, AC], f32)
        nc.sync.dma_start(out=wv_sb[:].rearrange("p (c o) -> p c o", c=AC),
                          in_=Wv.rearrange("(c p) o -> p c o", p=P))
        bk_sb = sbc.tile([P, AC], f32)
        nc.sync.dma_start(out=bk_sb[:], in_=bk.rearrange("(c p) -> p c", p=P))
        bv_sb = sbc.tile([1, 1], f32)
        nc.sync.dma_start(out=bv_sb[:], in_=bv[None, :])
        bq_row = sbc.tile([1, A], f32)
        nc.sync.dma_start(out=bq_row[:], in_=bq[None, :])

        # WkT [a-part, (c h)]: transpose Wk once on the PE (two halves to
        # halve SBUF staging)
        WkT_sb = sbc.tile([P, AC * H], f32)
        if True:
            for half in range(2):
                Wk_sb = sbkey.tile([P, NSUB * H], f32, tag="kg",
                                 name=f"wkstage{half}")
                nc.sync.dma_start(
                    out=Wk_sb[:, 0:(HJ // 2) * A]
                    .rearrange("p (j a) -> p j a", j=HJ // 2),
                    in_=Wk[half * (H // 2):(half + 1) * (H // 2), :]
                    .rearrange("(j p) a -> p j a", p=P))
                for c in range(AC):
                    wkt_ps = ps_small.tile([P, (HJ // 2) * P], f32, tag="small")
                    for jl in range(HJ // 2):
                        nc.tensor.transpose(
                            wkt_ps[:, jl * P:(jl + 1) * P],
                            Wk_sb[:, jl * A + c * P: jl * A + (c + 1) * P],
                            ident[:])
                    dst = WkT_sb[:, c * H + half * (H // 2):
                                 c * H + (half + 1) * (H // 2)]
                    if c % 2 == 0:
                        nc.scalar.copy(dst, wkt_ps[:])
                    else:
                        nc.vector.tensor_copy(dst, wkt_ps[:])

        emit_prep(0)
        emit_query(1)

        # u = Wk @ Wv[:,0]  as a row [1, H]
        u_sb = sbc.tile([1, H], f32)
        for half in range(2):
            u_ps = ps_small.tile([1, A], f32, tag="small")
            for c in range(AC):
                nc.tensor.matmul(
                    u_ps[:], wv_sb[:, c:c + 1],
                    WkT_sb[:, c * H + half * A: c * H + (half + 1) * A],
                    start=(c == 0), stop=(c == AC - 1))
            nc.scalar.copy(u_sb[:, half * A:(half + 1) * A], u_ps[:])

        # cv = bk . Wv + bv
        junk4 = sbsm.tile([P, AC], f32, tag="tiny2")
        cvcol = sbsm.tile([P, 1], f32, tag="tiny3")
        nc.vector.scalar_tensor_tensor(out=junk4[:], in0=bk_sb[:], scalar=1.0,
                                       in1=wv_sb[:], op0=mybir.AluOpType.mult,
                                       op1=mybir.AluOpType.mult, accum_out=cvcol[:])
        cv_ps = ps_small.tile([1, 1], f32, tag="small")
        nc.tensor.matmul(cv_ps[:], cvcol[:], ones[:], start=True, stop=True)
        cv_sb = sbc.tile([1, 1], f32)
        nc.vector.tensor_tensor(out=cv_sb[:], in0=cv_ps[:], in1=bv_sb[:],
                                op=mybir.AluOpType.add)


        wvbc_box[0] = sbw.tile([P, H], f32, tag="wvbc", bufs=1, name="wvbc")
        nc.gpsimd.partition_broadcast(wvbc_box[0][:], u_sb[:], channels=P)


        for b in range(BPC):
            for g in range(NG):
                emit_key_granule(b, g)
                if g == 0:
                    if b + 1 < BPC:
                        emit_prep(b + 1)
                    if b >= 1:
                        emit_softmax(b - 1)
                elif g == 1:
                    if b >= 1:
                        if is_kw(b - 1):
                            emit_kw(b - 1)
                        else:
                            emit_fin3(b - 1)
                elif g == 2:
                    if b >= 2 and is_kw(b - 2):
                        emit_fin(b - 2)
                if b + 2 < BPC:
                    emit_query_granule(b + 2, 2 * g)
                    emit_query_granule(b + 2, 2 * g + 1)
        if is_kw(BPC - 2):
            emit_fin(BPC - 2)
        emit_softmax(BPC - 1)
        if is_kw(BPC - 1):
            emit_kw(BPC - 1)
            emit_fin(BPC - 1)
        else:
            emit_fin3(BPC - 1)


def _shard(query, key, shared):
    in_maps = []
    for c in range(N_CORES):
        sl = slice(c * BPC, (c + 1) * BPC)
        m = {"query": np.ascontiguousarray(query[sl]),
             "key": np.ascontiguousarray(key[sl])}
        m.update(shared)
        in_maps.append(m)
    return in_maps


def _make_in_maps(inputs):
    query = np.ascontiguousarray(np.asarray(inputs["query"], dtype=np.float32))
    key = np.ascontiguousarray(np.asarray(inputs["key"], dtype=np.float32))
    shared = {k: np.ascontiguousarray(np.asarray(inputs[k], dtype=np.float32))
              for k in ("Wq", "bq", "Wk", "bk", "Wv", "bv")}
    return _shard(query, key, shared)


def kernel(**inputs):
    if "nc" not in _CACHE:
        _CACHE["nc"] = build_bass()
    nc = _CACHE["nc"]
    in_maps = _make_in_maps(inputs)
    res = run_bass_kernel_spmd(nc, in_maps, list(range(N_CORES)))
    outs = [res.results[c]["out"] for c in range(N_CORES)]
    return np.concatenate(outs, axis=0).astype(np.float32)


if __name__ == "__main__":
    rng = np.random.default_rng(0)
    ins = {
        "query": rng.standard_normal((B, LQ, H), dtype=np.float32),
        "key": rng.standard_normal((B, LK, H), dtype=np.float32),
        "Wq": (rng.standard_normal((H, A), dtype=np.float32) / np.sqrt(H)).astype(np.float32),
        "bq": np.zeros((A,), np.float32),
        "Wk": (rng.standard_normal((H, A), dtype=np.float32) / np.sqrt(H)).astype(np.float32),
        "bk": np.zeros((A,), np.float32),
        "Wv": (rng.standard_normal((A, 1), dtype=np.float32) / np.sqrt(A)).astype(np.float32),
        "bv": np.zeros((1,), np.float32),
    }
    x = kernel(**ins)
    print("kernel out:", x[:8, 0])


# revision 31
# speedup vs baseline: 1.0350x; 1.0350x over previous
"""Trainium2 Bass kernel for nn_Attention_2 (B=32, LQ=LK=2048, H=1024, A=512).

Math: the reference's softmax is over sum_q(Qp @ Kp^T); the q-sum
distributes through the matmuls so the [B, LQ, LK] score tensor never
exists:

  qs[b]   = sum_q query[b,q,:]                      [H]
  qp[b]   = qs[b] @ Wq + LQ*bq                      [A]
  t[b]    = qp[b] @ Wk^T                            [H]
  s[b,k]  = key[b,k,:] . t[b]     (+ const, cancels in softmax)
  e[b,k]  = exp(s[b,k] - max_k s)
  x[b]    = (sum_k e[b,k] * key[b,k,:]) . u / sum_k e[b,k] + cv
            where u = Wk @ Wv[:,0],  cv = bk.Wv + bv

The last line folds the per-key value v_k = key.u + cv through the
softmax combine, so no per-key v dot products are needed: the weighted
key sum is a post-softmax PE matmul (bf16 is plenty: only the relative
weight of the top few keys matters, tolerance is 2e-2).

Sharding: data-parallel over batch, 4 batches per core, 8 cores.
Per core the only heavy work is streaming query+key (64MB) from HBM as
plain 2MB granule DMAs (query on the scalar queue, key on the sync
queue — accumulate-DMAs run ~1.6x slower per engine, so query is
reduced on-chip instead):
  - query granules: gpsimd elementwise adds into acc[b], then 8 tiny
    PE ones-matmuls fold acc -> qsT columns
  - key subtiles: fp32 DVE fused mul+reduce vs broadcast t[b] (scores
    must be fp32: top-2 logit gaps ~1-4 at |s|~1000), plus a scalar
    engine bf16 cast of each subtile held for the post-softmax kw
    matmul on the PE.
"""
import numpy as np

import concourse.bass as bass
import concourse.bacc as bacc
import concourse.tile as tile
from concourse import mybir
from concourse import bass_isa
from concourse.bass_utils import run_bass_kernel_spmd

N_CORES = 8
B, LQ, LK, H, A = 32, 2048, 2048, 1024, 512
BPC = B // N_CORES          # batches per core
P = 128
f32 = mybir.dt.float32
bf16 = mybir.dt.bfloat16
NG = 4                      # 2MB key granules per batch
GK = LK // NG               # 512 rows per granule
NSUB = GK // P              # 4 subtiles per granule
NGQ = 4                     # 2MB query granules per batch
GQ = LQ // NGQ              # 512 rows per query granule
QSUB = GQ // P              # 4 subtiles per query granule
NKW = 2                     # batches using the PE kw route (rest: DVE v-dots)
NKT = LK // P               # 16 subtiles per batch
HJ = H // P                 # 8 h-chunks
AC = A // P                 # 4 a-chunks

_CACHE = {}
import os as _os
KG_BUFS = int(_os.environ.get("KG_BUFS", "2"))
QG_BUFS = int(_os.environ.get("QG_BUFS", "2"))


def build_bass(repeat=1, variant="full"):
    nc = bacc.Bacc(None, target_bir_lowering=False, debug=False)

    query = nc.dram_tensor("query", [BPC, LQ, H], f32, kind="ExternalInput").ap()
    key = nc.dram_tensor("key", [BPC, LK, H], f32, kind="ExternalInput").ap()
    Wq = nc.dram_tensor("Wq", [H, A], f32, kind="ExternalInput").ap()
    bq = nc.dram_tensor("bq", [A], f32, kind="ExternalInput").ap()
    Wk = nc.dram_tensor("Wk", [H, A], f32, kind="ExternalInput").ap()
    bk = nc.dram_tensor("bk", [A], f32, kind="ExternalInput").ap()
    Wv = nc.dram_tensor("Wv", [A, 1], f32, kind="ExternalInput").ap()
    bv = nc.dram_tensor("bv", [1], f32, kind="ExternalInput").ap()
    out = nc.dram_tensor("out", [BPC, 1], f32, kind="ExternalOutput").ap()

    with tile.TileContext(nc) as tc:
        for _ in range(repeat):
            _build_body(nc, tc, query, key, Wq, bq, Wk, bk, Wv, bv, out,
                        variant=variant)
    nc.compile()
    return nc


def _build_body(nc, tc, query, key, Wq, bq, Wk, bk, Wv, bv, out, variant="full"):
    from contextlib import ExitStack
    ctx = ExitStack()
    with ctx:
        sbc = ctx.enter_context(tc.tile_pool(name="sbc", bufs=1))
        sbqa = ctx.enter_context(tc.tile_pool(name="sbqa", bufs=2))
        sbq = ctx.enter_context(tc.tile_pool(name="sbq", bufs=QG_BUFS))
        sbkey = ctx.enter_context(tc.tile_pool(name="sbkey", bufs=KG_BUFS))
        sbkbf = ctx.enter_context(tc.tile_pool(name="sbkbf", bufs=3))
        sbw = ctx.enter_context(tc.tile_pool(name="sbw", bufs=2))
        sbsm = ctx.enter_context(tc.tile_pool(name="sbsm", bufs=2))
        sbjunk = ctx.enter_context(tc.tile_pool(name="sbjunk", bufs=1, space="PSUM"))
        ps_small = ctx.enter_context(tc.tile_pool(name="ps_small", bufs=2, space="PSUM"))
        ps_kw = ctx.enter_context(tc.tile_pool(name="ps_kw", bufs=4, space="PSUM"))

        if variant == "dma":
            # pure-DMA roofline probe: all loads, no compute
            for b in range(BPC):
                for g in range(NGQ):
                    qt = sbq.tile([P, QSUB * H], f32, tag="qg")
                    nc.scalar.dma_start(
                        out=qt[:].rearrange("p (n h) -> p n h", n=QSUB),
                        in_=query[b, g * GQ:(g + 1) * GQ, :]
                        .rearrange("(n p) h -> p n h", p=P))
                for g in range(NG):
                    kt = sbkey.tile([P, NSUB * H], f32, tag="kg")
                    nc.sync.dma_start(
                        out=kt[:].rearrange("p (n h) -> p n h", n=NSUB),
                        in_=key[b, g * GK:(g + 1) * GK, :]
                        .rearrange("(n p) h -> p n h", p=P))
            xz = sbsm.tile([1, 1], f32, tag="x")
            nc.vector.memset(xz[:], 0.0)
            for b in range(BPC):
                nc.sync.dma_start(out=out[b:b + 1, :], in_=xz[:])
            return

        # ---------------- constants ----------------
        ident = sbc.tile([P, P], f32)
        colidx = sbsm.tile([P, P], f32, tag="small", bufs=1)
        rowidx = sbsm.tile([P, 1], f32, tag="tiny")
        nc.gpsimd.iota(colidx[:], pattern=[[1, P]], base=0, channel_multiplier=0,
                       allow_small_or_imprecise_dtypes=True)
        nc.gpsimd.iota(rowidx[:], pattern=[[0, 1]], base=0, channel_multiplier=1,
                       allow_small_or_imprecise_dtypes=True)
        nc.vector.tensor_scalar(out=ident[:], in0=colidx[:], scalar1=rowidx[:],
                                scalar2=None, op0=mybir.AluOpType.is_equal)

        ones = sbc.tile([P, 1], f32)            # column of 1s
        nc.vector.memset(ones[:], 1.0)
        ones_k1 = sbc.tile([1, P], f32)         # single-partition row of 1s
        nc.vector.memset(ones_k1[:], 1.0)
        inv128 = sbc.tile([P, 1], f32)          # column of 1/128
        nc.vector.memset(inv128[:], 1.0 / P)
        lqconst = sbc.tile([1, 1], f32)         # LQ (for the bq bias matmul)
        nc.vector.memset(lqconst[:], float(LQ))

        # ---------------- per-batch pieces ----------------
        qacc = [None] * BPC         # pinned first query granule = accumulator
        qacc2 = [None] * BPC        # second accumulation target (g2+g3)
        wqbcs = [None] * BPC
        state = {}
        wvbc_box = [None]

        def is_kw(b):
            return b < NKW

        def emit_query_granule(b, g):
            """2MB query granule, plain load (scalar/gpsimd queues).
            Granule 0 is pinned as the accumulator; 1-3 added by DVE."""
            if g == 0:
                qt = sbqa.tile([P, QSUB * H], f32, tag="qa", name=f"qa{b}")
                qacc[b] = qt
            else:
                qt = sbq.tile([P, QSUB * H], f32, tag="qg", name=f"q{b}_{g}")
            qeng = {0: nc.scalar, 1: nc.gpsimd, 2: nc.scalar,
                    3: nc.sync}[g]
            qeng.dma_start(
                out=qt[:].rearrange("p (n h) -> p n h", n=QSUB),
                in_=query[b, g * GQ:(g + 1) * GQ, :]
                .rearrange("(n p) h -> p n h", p=P))
            if g > 0:
                acc = qacc[b]
                nc.vector.tensor_tensor(out=acc[:], in0=acc[:], in1=qt[:],
                                        op=mybir.AluOpType.add)
                if g == NGQ - 1:
                    # fold the 4 subtile blocks down to acc[:, 0:H]
                    nc.vector.tensor_tensor(out=acc[:, 0:2 * H],
                                            in0=acc[:, 0:2 * H],
                                            in1=acc[:, 2 * H:4 * H],
                                            op=mybir.AluOpType.add)
                    nc.vector.tensor_tensor(out=acc[:, 0:H], in0=acc[:, 0:H],
                                            in1=acc[:, H:2 * H],
                                            op=mybir.AluOpType.add)

        def emit_query(b):
            for g in range(NGQ):
                emit_query_granule(b, g)

        def emit_prep(b):
            """qsT -> qp -> t -> broadcast t across partitions."""
            acc = qacc[b]
            # qsT[m, j] = sum over the 4 subtile blocks and partitions
            qsT_ps = ps_small.tile([P, HJ], f32, tag="small")
            for j in range(HJ):
                nc.tensor.matmul(qsT_ps[:, j:j + 1],
                                 acc[:, j * P:(j + 1) * P],
                                 ones[:], start=True, stop=True)
            qsT_sb = sbsm.tile([P, HJ], f32, tag="qsT", bufs=1)
            nc.scalar.copy(qsT_sb[:], qsT_ps[:])

            # qp = qs @ Wq  [1, A]
            qp_ps = ps_small.tile([1, A], f32, tag="small")
            for j in range(HJ):
                nc.tensor.matmul(qp_ps[:], qsT_sb[:, j:j + 1],
                                 Wq_sb[:, j * A:(j + 1) * A],
                                 start=(j == 0), stop=False)
            nc.tensor.matmul(qp_ps[:], lqconst[:], bq_row[:],
                             start=False, stop=True)
            qp2 = sbsm.tile([1, A], f32, tag="qp2", bufs=1)
            nc.scalar.copy(qp2[:], qp_ps[:])
            qp2bc = sbsm.tile([P, A], f32, tag="qp2bc", bufs=1)
            nc.gpsimd.partition_broadcast(qp2bc[:], qp2[:], channels=P)
            qpT_ps = ps_small.tile([P, AC], f32, tag="small")
            for c in range(AC):
                nc.tensor.matmul(qpT_ps[:, c:c + 1],
                                 qp2bc[:, c * P:(c + 1) * P], inv128[:],
                                 start=True, stop=True)
            qpT_sb = sbsm.tile([P, AC], f32, tag="qpT", bufs=1)
            nc.scalar.copy(qpT_sb[:], qpT_ps[:])

            # t = qp2 @ Wk^T  [1, H] in two halves
            t_sb = sbsm.tile([1, H], f32, tag="t", bufs=1)
            for half in range(2):
                t_ps = ps_small.tile([1, A], f32, tag="small")
                for c in range(AC):
                    nc.tensor.matmul(
                        t_ps[:], qpT_sb[:, c:c + 1],
                        WkT_sb[:, c * H + half * A: c * H + (half + 1) * A],
                        start=(c == 0), stop=(c == AC - 1))
                nc.scalar.copy(t_sb[:, half * A:(half + 1) * A], t_ps[:])

            wqbc = sbw.tile([P, H], f32, tag="wqbc", name=f"wqbc{b}")
            nc.gpsimd.partition_broadcast(wqbc[:], t_sb[:], channels=P)
            wqbcs[b] = wqbc

        def emit_key_granule(b, g):
            """One 2MB key granule: fp32 s-dots; bf16 casts (kw batches) or
            fp32 v-dots (classic batches)."""
            st = state.setdefault(b, {})
            if g == 0:
                st["sdve"] = sbsm.tile([P, NKT], f32, tag="sdve",
                                       name=f"sdve{b}")
                if not is_kw(b):
                    st["vdve"] = sbsm.tile([P, NKT], f32, tag="vdve",
                                           name=f"vdve{b}")
            if is_kw(b) and g % 2 == 0:
                st.setdefault("kbf", []).append(
                    sbkbf.tile([P, (NKT // 2) * H], bf16, tag="kbf",
                               name=f"kbf{b}_{g // 2}"))
            sdve = st["sdve"]
            wqbc = wqbcs[b]
            kt = sbkey.tile([P, NSUB * H], f32, tag="kg", name=f"k{b}_{g}")
            keng = nc.sync if g % 2 == 0 else nc.gpsimd
            keng.dma_start(
                out=kt[:].rearrange("p (n h) -> p n h", n=NSUB),
                in_=key[b, g * GK:(g + 1) * GK, :]
                .rearrange("(n p) h -> p n h", p=P))
            for n in range(NSUB):
                t = g * NSUB + n
                kv = kt[:, n * H:(n + 1) * H]
                junk = sbjunk.tile([P, H], f32, tag="junk")
                nc.vector.scalar_tensor_tensor(
                    out=junk[:], in0=kv, scalar=1.0, in1=wqbc[:],
                    op0=mybir.AluOpType.mult, op1=mybir.AluOpType.mult,
                    accum_out=sdve[:, t:t + 1])
                if is_kw(b):
                    kbf = st["kbf"][g // 2]
                    toff = t - (g // 2) * (NKT // 2)
                    nc.scalar.copy(kbf[:, toff * H:(toff + 1) * H], kv)
                else:
                    junk2 = sbjunk.tile([P, H], f32, tag="junk")
                    nc.vector.scalar_tensor_tensor(
                        out=junk2[:], in0=kv, scalar=1.0, in1=wvbc_box[0][:],
                        op0=mybir.AluOpType.mult, op1=mybir.AluOpType.mult,
                        accum_out=st["vdve"][:, t:t + 1])

        def emit_softmax(b):
            st = state[b]
            sdve = st["sdve"]
            m1 = sbsm.tile([P, 1], f32, tag="m1", bufs=1)
            nc.vector.reduce_max(m1[:], sdve[:], axis=mybir.AxisListType.X)
            mT_ps = ps_small.tile([1, P], f32, tag="small")
            nc.tensor.transpose(mT_ps[:], m1[:], ident[:])
            mT_sb = sbsm.tile([1, P], f32, tag="mT", bufs=1)
            nc.vector.tensor_copy(mT_sb[:], mT_ps[:])
            gmax = sbsm.tile([1, 1], f32, tag="gmax")
            nc.vector.reduce_max(gmax[:], mT_sb[:], axis=mybir.AxisListType.X)
            ng_ps = ps_small.tile([P, 1], f32, tag="small")
            nc.tensor.matmul(ng_ps[:], ones_k1[:], gmax[:], start=True, stop=True)
            ngm = sbsm.tile([P, 1], f32, tag="ngm")
            nc.vector.tensor_scalar_mul(ngm[:], ng_ps[:], -1.0)
            e128 = sbsm.tile([P, NKT], f32, tag="e128", name=f"e128_{b}")
            erow = sbsm.tile([P, 1], f32, tag="erow", name=f"erow{b}")
            nc.scalar.activation(e128[:], sdve[:],
                                 mybir.ActivationFunctionType.Exp,
                                 bias=ngm[:], scale=1.0, accum_out=erow[:])
            st["e128"] = e128
            st["erow"] = erow

        def emit_kw(b):
            st = state[b]
            e_bf = sbsm.tile([P, NKT], bf16, tag="ebf", name=f"ebf{b}")
            nc.scalar.copy(e_bf[:], st["e128"][:])
            kw_ps = [ps_kw.tile([1, A], f32, tag="kw", name=f"kw{b}_{h}")
                     for h in range(2)]
            for t in range(NKT):
                kbf = st["kbf"][t // (NKT // 2)]
                toff = t % (NKT // 2)
                for half in range(2):
                    nc.tensor.matmul(
                        kw_ps[half][:], e_bf[:, t:t + 1],
                        kbf[:, toff * H + half * A: toff * H + (half + 1) * A],
                        start=(t == 0), stop=(t == NKT - 1))
            den_ps = ps_small.tile([1, 1], f32, tag="small")
            nc.tensor.matmul(den_ps[:], st["erow"][:], ones[:],
                             start=True, stop=True)
            den_sb = sbsm.tile([1, 1], f32, tag="den", name=f"den{b}")
            nc.vector.tensor_copy(den_sb[:], den_ps[:])
            st["kw_ps"] = kw_ps
            st["den_sb"] = den_sb

        def emit_fin(b):
            """Final combine for a kw batch; reads kw straight from PSUM."""
            st = state[b]
            kw_ps = st["kw_ps"]
            junk3 = sbjunk.tile([P, H], f32, tag="junk")
            nums = sbsm.tile([1, 2], f32, tag="nums")
            for half in range(2):
                nc.vector.scalar_tensor_tensor(
                    out=junk3[0:1, half * A:(half + 1) * A],
                    in0=kw_ps[half][:], scalar=1.0,
                    in1=u_sb[:, half * A:(half + 1) * A],
                    op0=mybir.AluOpType.mult, op1=mybir.AluOpType.mult,
                    accum_out=nums[:, half:half + 1])
            num = sbsm.tile([1, 1], f32, tag="num")
            nc.vector.tensor_tensor(out=num[:], in0=nums[:, 0:1],
                                    in1=nums[:, 1:2], op=mybir.AluOpType.add)
            _emit_final(num[:], st["den_sb"][:], b)

        def emit_fin3(b):
            """Combine for a classic (v-dot) batch."""
            st = state[b]
            junk5 = sbsm.tile([P, NKT], f32, tag="junk5", bufs=1)
            nrow = sbsm.tile([P, 1], f32, tag="nrow")
            nc.vector.scalar_tensor_tensor(
                out=junk5[:], in0=st["e128"][:], scalar=1.0, in1=st["vdve"][:],
                op0=mybir.AluOpType.mult, op1=mybir.AluOpType.mult,
                accum_out=nrow[:])
            dn_ps = ps_small.tile([1, 2], f32, tag="small")
            nc.tensor.matmul(dn_ps[:, 0:1], st["erow"][:], ones[:],
                             start=True, stop=True)
            nc.tensor.matmul(dn_ps[:, 1:2], nrow[:], ones[:],
                             start=True, stop=True)
            dn = sbsm.tile([1, 2], f32, tag="dn")
            nc.vector.tensor_copy(dn[:], dn_ps[:])
            _emit_final(dn[:, 1:2], dn[:, 0:1], b)

        def _emit_final(num, den, b):
            rden = sbsm.tile([1, 1], f32, tag="rden")
            nc.vector.reciprocal(rden[:], den)
            x = sbsm.tile([1, 1], f32, tag="x")
            nc.vector.tensor_tensor(out=x[:], in0=num, in1=rden[:],
                                    op=mybir.AluOpType.mult)
            x2 = sbsm.tile([1, 1], f32, tag="x2")
            nc.vector.tensor_tensor(out=x2[:], in0=x[:], in1=cv_sb[:],
                                    op=mybir.AluOpType.add)
            nc.sync.dma_start(out=out[b:b + 1, :], in_=x2[:])

        # ---------------- pipeline ----------------
        emit_query(0)

        # Wq natural layout: [h-part, (j a)] ; chunk j at cols [j*A, (j+1)*A)
        Wq_sb = sbc.tile([P, HJ * A], f32)
        nc.sync.dma_start(out=Wq_sb[:].rearrange("p (j a) -> p j a", j=HJ),
                          in_=Wq.rearrange("(j p) a -> p j a", p=P))

        wv_sb = sbc.tile([P, AC], f32)
        nc.sync.dma_start(out=wv_sb[:].rearrange("p (c o) -> p c o", c=AC),
                          in_=Wv.rearrange("(c p) o -> p c o", p=P))
        bk_sb = sbc.tile([P, AC], f32)
        nc.sync.dma_start(out=bk_sb[:], in_=bk.rearrange("(c p) -> p c", p=P))
        bv_sb = sbc.tile([1, 1], f32)
        nc.sync.dma_start(out=bv_sb[:], in_=bv[None, :])
        bq_row = sbc.tile([1, A], f32)
        nc.sync.dma_start(out=bq_row[:], in_=bq[None, :])

        # WkT [a-part, (c h)]: transpose Wk once on the PE (two halves to
        # halve SBUF staging)
        WkT_sb = sbc.tile([P, AC * H], f32)
        if True:
            for half in range(2):
                Wk_sb = sbkey.tile([P, NSUB * H], f32, tag="kg",
                                 name=f"wkstage{half}")
                nc.sync.dma_start(
                    out=Wk_sb[:, 0:(HJ // 2) * A]
                    .rearrange("p (j a) -> p j a", j=HJ // 2),
                    in_=Wk[half * (H // 2):(half + 1) * (H // 2), :]
                    .rearrange("(j p) a -> p j a", p=P))
                for c in range(AC):
                    wkt_ps = ps_small.tile([P, (HJ // 2) * P], f32, tag="small")
                    for jl in range(HJ // 2):
                        nc.tensor.transpose(
                            wkt_ps[:, jl * P:(jl + 1) * P],
                            Wk_sb[:, jl * A + c * P: jl * A + (c + 1) * P],
                            ident[:])
                    dst = WkT_sb[:, c * H + half * (H // 2):
                                 c * H + (half + 1) * (H // 2)]
                    if c % 2 == 0:
                        nc.scalar.copy(dst, wkt_ps[:])
                    else:
                        nc.vector.tensor_copy(dst, wkt_ps[:])

        emit_prep(0)
        emit_query(1)

        # u = Wk @ Wv[:,0]  as a row [1, H]
        u_sb = sbc.tile([1, H], f32)
        for half in range(2):
            u_ps = ps_small.tile([1, A], f32, tag="small")
            for c in range(AC):
                nc.tensor.matmul(
                    u_ps[:], wv_sb[:, c:c + 1],
                    WkT_sb[:, c * H + half * A: c * H + (half + 1) * A],
                    start=(c == 0), stop=(c == AC - 1))
            nc.scalar.copy(u_sb[:, half * A:(half + 1) * A], u_ps[:])

        # cv = bk . Wv + bv
        junk4 = sbsm.tile([P, AC], f32, tag="tiny2")
        cvcol = sbsm.tile([P, 1], f32, tag="tiny3")
        nc.vector.scalar_tensor_tensor(out=junk4[:], in0=bk_sb[:], scalar=1.0,
                                       in1=wv_sb[:], op0=mybir.AluOpType.mult,
                                       op1=mybir.AluOpType.mult, accum_out=cvcol[:])
        cv_ps = ps_small.tile([1, 1], f32, tag="small")
        nc.tensor.matmul(cv_ps[:], cvcol[:], ones[:], start=True, stop=True)
        cv_sb = sbc.tile([1, 1], f32)
        nc.vector.tensor_tensor(out=cv_sb[:], in0=cv_ps[:], in1=bv_sb[:],
                                op=mybir.AluOpType.add)


        wvbc_box[0] = sbw.tile([P, H], f32, tag="wvbc", bufs=1, name="wvbc")
        nc.gpsimd.partition_broadcast(wvbc_box[0][:], u_sb[:], channels=P)


        for b in range(BPC):
            for g in range(NG):
                emit_key_granule(b, g)
                if g == 0:
                    if b + 1 < BPC:
                        emit_prep(b + 1)
                    if b + 2 < BPC:
                        emit_query_granule(b + 2, 0)
                    if b >= 1:
                        emit_softmax(b - 1)
                elif g == 1:
                    if b >= 1:
                        if is_kw(b - 1):
                            emit_kw(b - 1)
                        else:
                            emit_fin3(b - 1)
                    if b + 2 < BPC:
                        emit_query_granule(b + 2, 1)
                elif g == 2:
                    if b >= 2 and is_kw(b - 2):
                        emit_fin(b - 2)
                    if b + 2 < BPC:
                        emit_query_granule(b + 2, 2)
                else:
                    if b + 2 < BPC:
                        emit_query_granule(b + 2, 3)
        if is_kw(BPC - 2):
            emit_fin(BPC - 2)
        emit_softmax(BPC - 1)
        if is_kw(BPC - 1):
            emit_kw(BPC - 1)
            emit_fin(BPC - 1)
        else:
            emit_fin3(BPC - 1)


def _shard(query, key, shared):
    in_maps = []
    for c in range(N_CORES):
        sl = slice(c * BPC, (c + 1) * BPC)
        m = {"query": np.ascontiguousarray(query[sl]),
             "key": np.ascontiguousarray(key[sl])}
        m.update(shared)
        in_maps.append(m)
    return in_maps


def _make_in_maps(inputs):
    query = np.ascontiguousarray(np.asarray(inputs["query"], dtype=np.float32))
    key = np.ascontiguousarray(np.asarray(inputs["key"], dtype=np.float32))
    shared = {k: np.ascontiguousarray(np.asarray(inputs[k], dtype=np.float32))
              for k in ("Wq", "bq", "Wk", "bk", "Wv", "bv")}
    return _shard(query, key, shared)


def kernel(**inputs):
    if "nc" not in _CACHE:
        _CACHE["nc"] = build_bass()
    nc = _CACHE["nc"]
    in_maps = _make_in_maps(inputs)
    res = run_bass_kernel_spmd(nc, in_maps, list(range(N_CORES)))
    outs = [res.results[c]["out"] for c in range(N_CORES)]
    return np.concatenate(outs, axis=0).astype(np.float32)


if __name__ == "__main__":
    rng = np.random.default_rng(0)
    ins = {
        "query": rng.standard_normal((B, LQ, H), dtype=np.float32),
        "key": rng.standard_normal((B, LK, H), dtype=np.float32),
        "Wq": (rng.standard_normal((H, A), dtype=np.float32) / np.sqrt(H)).astype(np.float32),
        "bq": np.zeros((A,), np.float32),
        "Wk": (rng.standard_normal((H, A), dtype=np.float32) / np.sqrt(H)).astype(np.float32),
        "bk": np.zeros((A,), np.float32),
        "Wv": (rng.standard_normal((A, 1), dtype=np.float32) / np.sqrt(A)).astype(np.float32),
        "bv": np.zeros((1,), np.float32),
    }
    x = kernel(**ins)
    print("kernel out:", x[:8, 0])


# revision 33
# speedup vs baseline: 1.0737x; 1.0373x over previous
"""Trainium2 Bass kernel for nn_Attention_2 (B=32, LQ=LK=2048, H=1024, A=512).

Math: the reference's softmax is over sum_q(Qp @ Kp^T); the q-sum
distributes through the matmuls so the [B, LQ, LK] score tensor never
exists:

  qs[b]   = sum_q query[b,q,:]                      [H]
  qp[b]   = qs[b] @ Wq + LQ*bq                      [A]
  t[b]    = qp[b] @ Wk^T                            [H]
  s[b,k]  = key[b,k,:] . t[b]     (+ const, cancels in softmax)
  e[b,k]  = exp(s[b,k] - max_k s)
  x[b]    = (sum_k e[b,k] * key[b,k,:]) . u / sum_k e[b,k] + cv
            where u = Wk @ Wv[:,0],  cv = bk.Wv + bv

The last line folds the per-key value v_k = key.u + cv through the
softmax combine, so no per-key v dot products are needed: the weighted
key sum is a post-softmax PE matmul (bf16 is plenty: only the relative
weight of the top few keys matters, tolerance is 2e-2).

Sharding: data-parallel over batch, 4 batches per core, 8 cores.
Per core the only heavy work is streaming query+key (64MB) from HBM as
plain 2MB granule DMAs (query on the scalar queue, key on the sync
queue — accumulate-DMAs run ~1.6x slower per engine, so query is
reduced on-chip instead):
  - query granules: gpsimd elementwise adds into acc[b], then 8 tiny
    PE ones-matmuls fold acc -> qsT columns
  - key subtiles: fp32 DVE fused mul+reduce vs broadcast t[b] (scores
    must be fp32: top-2 logit gaps ~1-4 at |s|~1000), plus a scalar
    engine bf16 cast of each subtile held for the post-softmax kw
    matmul on the PE.
"""
import numpy as np

import concourse.bass as bass
import concourse.bacc as bacc
import concourse.tile as tile
from concourse import mybir
from concourse import bass_isa
from concourse.bass_utils import run_bass_kernel_spmd

N_CORES = 8
B, LQ, LK, H, A = 32, 2048, 2048, 1024, 512
BPC = B // N_CORES          # batches per core
P = 128
f32 = mybir.dt.float32
bf16 = mybir.dt.bfloat16
NG = 4                      # 2MB key granules per batch
GK = LK // NG               # 512 rows per granule
NSUB = GK // P              # 4 subtiles per granule
NGQ = 4                     # 2MB query granules per batch
GQ = LQ // NGQ              # 512 rows per query granule
QSUB = GQ // P              # 4 subtiles per query granule
NKW = 2                     # batches using the PE kw route (rest: DVE v-dots)
NKT = LK // P               # 16 subtiles per batch
HJ = H // P                 # 8 h-chunks
AC = A // P                 # 4 a-chunks

_CACHE = {}
import os as _os
KG_BUFS = int(_os.environ.get("KG_BUFS", "2"))
QG_BUFS = int(_os.environ.get("QG_BUFS", "2"))


def build_bass(repeat=1, variant="full"):
    nc = bacc.Bacc(None, target_bir_lowering=False, debug=False)

    query = nc.dram_tensor("query", [BPC, LQ, H], f32, kind="ExternalInput").ap()
    key = nc.dram_tensor("key", [BPC, LK, H], f32, kind="ExternalInput").ap()
    Wq = nc.dram_tensor("Wq", [H, A], f32, kind="ExternalInput").ap()
    bq = nc.dram_tensor("bq", [A], f32, kind="ExternalInput").ap()
    Wk = nc.dram_tensor("Wk", [H, A], f32, kind="ExternalInput").ap()
    bk = nc.dram_tensor("bk", [A], f32, kind="ExternalInput").ap()
    Wv = nc.dram_tensor("Wv", [A, 1], f32, kind="ExternalInput").ap()
    bv = nc.dram_tensor("bv", [1], f32, kind="ExternalInput").ap()
    out = nc.dram_tensor("out", [BPC, 1], f32, kind="ExternalOutput").ap()

    with tile.TileContext(nc) as tc:
        for _ in range(repeat):
            _build_body(nc, tc, query, key, Wq, bq, Wk, bk, Wv, bv, out,
                        variant=variant)
    nc.compile()
    return nc


def _build_body(nc, tc, query, key, Wq, bq, Wk, bk, Wv, bv, out, variant="full"):
    from contextlib import ExitStack
    ctx = ExitStack()
    with ctx:
        sbc = ctx.enter_context(tc.tile_pool(name="sbc", bufs=1))
        sbqa = ctx.enter_context(tc.tile_pool(name="sbqa", bufs=2))
        sbq = ctx.enter_context(tc.tile_pool(name="sbq", bufs=QG_BUFS))
        sbkey = ctx.enter_context(tc.tile_pool(name="sbkey", bufs=KG_BUFS))
        sbkbf = ctx.enter_context(tc.tile_pool(name="sbkbf", bufs=3))
        sbw = ctx.enter_context(tc.tile_pool(name="sbw", bufs=2))
        sbsm = ctx.enter_context(tc.tile_pool(name="sbsm", bufs=2))
        sbjunk = ctx.enter_context(tc.tile_pool(name="sbjunk", bufs=1, space="PSUM"))
        ps_small = ctx.enter_context(tc.tile_pool(name="ps_small", bufs=2, space="PSUM"))
        ps_kw = ctx.enter_context(tc.tile_pool(name="ps_kw", bufs=4, space="PSUM"))

        if variant == "dma":
            # pure-DMA roofline probe: all loads, no compute
            for b in range(BPC):
                for g in range(NGQ):
                    qt = sbq.tile([P, QSUB * H], f32, tag="qg")
                    nc.scalar.dma_start(
                        out=qt[:].rearrange("p (n h) -> p n h", n=QSUB),
                        in_=query[b, g * GQ:(g + 1) * GQ, :]
                        .rearrange("(n p) h -> p n h", p=P))
                for g in range(NG):
                    kt = sbkey.tile([P, NSUB * H], f32, tag="kg")
                    nc.sync.dma_start(
                        out=kt[:].rearrange("p (n h) -> p n h", n=NSUB),
                        in_=key[b, g * GK:(g + 1) * GK, :]
                        .rearrange("(n p) h -> p n h", p=P))
            xz = sbsm.tile([1, 1], f32, tag="x")
            nc.vector.memset(xz[:], 0.0)
            for b in range(BPC):
                nc.sync.dma_start(out=out[b:b + 1, :], in_=xz[:])
            return

        # ---------------- constants ----------------
        ident = sbc.tile([P, P], f32)
        colidx = sbsm.tile([P, P], f32, tag="small", bufs=1)
        rowidx = sbsm.tile([P, 1], f32, tag="tiny")
        nc.gpsimd.iota(colidx[:], pattern=[[1, P]], base=0, channel_multiplier=0,
                       allow_small_or_imprecise_dtypes=True)
        nc.gpsimd.iota(rowidx[:], pattern=[[0, 1]], base=0, channel_multiplier=1,
                       allow_small_or_imprecise_dtypes=True)
        nc.vector.tensor_scalar(out=ident[:], in0=colidx[:], scalar1=rowidx[:],
                                scalar2=None, op0=mybir.AluOpType.is_equal)

        ones = sbc.tile([P, 1], f32)            # column of 1s
        nc.vector.memset(ones[:], 1.0)
        ones_k1 = sbc.tile([1, P], f32)         # single-partition row of 1s
        nc.vector.memset(ones_k1[:], 1.0)
        inv128 = sbc.tile([P, 1], f32)          # column of 1/128
        nc.vector.memset(inv128[:], 1.0 / P)
        lqconst = sbc.tile([1, 1], f32)         # LQ (for the bq bias matmul)
        nc.vector.memset(lqconst[:], float(LQ))

        # ---------------- per-batch pieces ----------------
        qacc = [None] * BPC         # pinned first query granule = accumulator
        qacc2 = [None] * BPC        # second accumulation target (g2+g3)
        wqbcs = [None] * BPC
        state = {}
        wvbc_box = [None]

        def is_kw(b):
            return b < NKW

        def emit_query_granule(b, g):
            """2MB query granule, plain load (scalar/gpsimd queues).
            Granule 0 is pinned as the accumulator; 1-3 added by DVE."""
            if g == 0:
                qt = sbqa.tile([P, QSUB * H], f32, tag="qa", name=f"qa{b}")
                qacc[b] = qt
            else:
                qt = sbq.tile([P, QSUB * H], f32, tag="qg", name=f"q{b}_{g}")
            qeng = {0: nc.scalar, 1: nc.gpsimd, 2: nc.scalar,
                    3: nc.sync}[g]
            qeng.dma_start(
                out=qt[:].rearrange("p (n h) -> p n h", n=QSUB),
                in_=query[b, g * GQ:(g + 1) * GQ, :]
                .rearrange("(n p) h -> p n h", p=P))
            if g > 0:
                acc = qacc[b]
                nc.vector.tensor_tensor(out=acc[:], in0=acc[:], in1=qt[:],
                                        op=mybir.AluOpType.add)
                if g == NGQ - 1:
                    # fold the 4 subtile blocks down to acc[:, 0:H]
                    nc.vector.tensor_tensor(out=acc[:, 0:2 * H],
                                            in0=acc[:, 0:2 * H],
                                            in1=acc[:, 2 * H:4 * H],
                                            op=mybir.AluOpType.add)
                    nc.vector.tensor_tensor(out=acc[:, 0:H], in0=acc[:, 0:H],
                                            in1=acc[:, H:2 * H],
                                            op=mybir.AluOpType.add)

        def emit_query(b):
            for g in range(NGQ):
                emit_query_granule(b, g)

        def emit_prep(b):
            """qsT -> qp -> t -> broadcast t across partitions."""
            acc = qacc[b]
            # qsT[m, j] = sum over the 4 subtile blocks and partitions
            qsT_ps = ps_small.tile([P, HJ], f32, tag="small")
            for j in range(HJ):
                nc.tensor.matmul(qsT_ps[:, j:j + 1],
                                 acc[:, j * P:(j + 1) * P],
                                 ones[:], start=True, stop=True)
            qsT_sb = sbsm.tile([P, HJ], f32, tag="qsT", bufs=1)
            nc.scalar.copy(qsT_sb[:], qsT_ps[:])

            # qp = qs @ Wq  [1, A]
            qp_ps = ps_small.tile([1, A], f32, tag="small")
            for j in range(HJ):
                nc.tensor.matmul(qp_ps[:], qsT_sb[:, j:j + 1],
                                 Wq_sb[:, j * A:(j + 1) * A],
                                 start=(j == 0), stop=False)
            nc.tensor.matmul(qp_ps[:], lqconst[:], bq_row[:],
                             start=False, stop=True)
            qp2 = sbsm.tile([1, A], f32, tag="qp2", bufs=1)
            nc.scalar.copy(qp2[:], qp_ps[:])
            qp2bc = sbsm.tile([P, A], f32, tag="qp2bc", bufs=1)
            nc.gpsimd.partition_broadcast(qp2bc[:], qp2[:], channels=P)
            qpT_ps = ps_small.tile([P, AC], f32, tag="small")
            for c in range(AC):
                nc.tensor.matmul(qpT_ps[:, c:c + 1],
                                 qp2bc[:, c * P:(c + 1) * P], inv128[:],
                                 start=True, stop=True)
            qpT_sb = sbsm.tile([P, AC], f32, tag="qpT", bufs=1)
            nc.scalar.copy(qpT_sb[:], qpT_ps[:])

            # t = qp2 @ Wk^T  [1, H] in two halves
            t_sb = sbsm.tile([1, H], f32, tag="t", bufs=1)
            for half in range(2):
                t_ps = ps_small.tile([1, A], f32, tag="small")
                for c in range(AC):
                    nc.tensor.matmul(
                        t_ps[:], qpT_sb[:, c:c + 1],
                        WkT_sb[:, c * H + half * A: c * H + (half + 1) * A],
                        start=(c == 0), stop=(c == AC - 1))
                nc.scalar.copy(t_sb[:, half * A:(half + 1) * A], t_ps[:])

            wqbc = sbw.tile([P, H], f32, tag="wqbc", name=f"wqbc{b}")
            nc.gpsimd.partition_broadcast(wqbc[:], t_sb[:], channels=P)
            wqbcs[b] = wqbc

        def emit_key_granule(b, g):
            """One 2MB key granule: fp32 s-dots; bf16 casts (kw batches) or
            fp32 v-dots (classic batches)."""
            st = state.setdefault(b, {})
            if g == 0:
                st["sdve"] = sbsm.tile([P, NKT], f32, tag="sdve",
                                       name=f"sdve{b}")
                if not is_kw(b):
                    st["vdve"] = sbsm.tile([P, NKT], f32, tag="vdve",
                                           name=f"vdve{b}")
            if is_kw(b) and g % 2 == 0:
                st.setdefault("kbf", []).append(
                    sbkbf.tile([P, (NKT // 2) * H], bf16, tag="kbf",
                               name=f"kbf{b}_{g // 2}"))
            sdve = st["sdve"]
            wqbc = wqbcs[b]
            # late batches recycle pools that are dead by then: b2's tail
            # granules use the kbf pool (freed by kw(1)), b3's head uses the
            # query streaming pool (queries all loaded) -- deeper prefetch
            if b == BPC - 2 and g >= 2:
                kt = sbkbf.tile([P, NSUB * H], f32, tag="kbf", name=f"k{b}_{g}")
            elif b == BPC - 1 and g < 2:
                kt = sbq.tile([P, NSUB * H], f32, tag="qg", name=f"k{b}_{g}")
            else:
                kt = sbkey.tile([P, NSUB * H], f32, tag="kg", name=f"k{b}_{g}")
            keng = nc.sync if g % 2 == 0 else nc.gpsimd
            keng.dma_start(
                out=kt[:].rearrange("p (n h) -> p n h", n=NSUB),
                in_=key[b, g * GK:(g + 1) * GK, :]
                .rearrange("(n p) h -> p n h", p=P))
            for n in range(NSUB):
                t = g * NSUB + n
                kv = kt[:, n * H:(n + 1) * H]
                junk = sbjunk.tile([P, H], f32, tag="junk")
                nc.vector.scalar_tensor_tensor(
                    out=junk[:], in0=kv, scalar=1.0, in1=wqbc[:],
                    op0=mybir.AluOpType.mult, op1=mybir.AluOpType.mult,
                    accum_out=sdve[:, t:t + 1])
                if is_kw(b):
                    kbf = st["kbf"][g // 2]
                    toff = t - (g // 2) * (NKT // 2)
                    nc.scalar.copy(kbf[:, toff * H:(toff + 1) * H], kv)
                else:
                    junk2 = sbjunk.tile([P, H], f32, tag="junk")
                    nc.vector.scalar_tensor_tensor(
                        out=junk2[:], in0=kv, scalar=1.0, in1=wvbc_box[0][:],
                        op0=mybir.AluOpType.mult, op1=mybir.AluOpType.mult,
                        accum_out=st["vdve"][:, t:t + 1])

        def emit_softmax(b):
            st = state[b]
            sdve = st["sdve"]
            m1 = sbsm.tile([P, 1], f32, tag="m1", bufs=1)
            nc.vector.reduce_max(m1[:], sdve[:], axis=mybir.AxisListType.X)
            mT_ps = ps_small.tile([1, P], f32, tag="small")
            nc.tensor.transpose(mT_ps[:], m1[:], ident[:])
            mT_sb = sbsm.tile([1, P], f32, tag="mT", bufs=1)
            nc.vector.tensor_copy(mT_sb[:], mT_ps[:])
            gmax = sbsm.tile([1, 1], f32, tag="gmax")
            nc.vector.reduce_max(gmax[:], mT_sb[:], axis=mybir.AxisListType.X)
            ng_ps = ps_small.tile([P, 1], f32, tag="small")
            nc.tensor.matmul(ng_ps[:], ones_k1[:], gmax[:], start=True, stop=True)
            ngm = sbsm.tile([P, 1], f32, tag="ngm")
            nc.vector.tensor_scalar_mul(ngm[:], ng_ps[:], -1.0)
            e128 = sbsm.tile([P, NKT], f32, tag="e128", name=f"e128_{b}")
            erow = sbsm.tile([P, 1], f32, tag="erow", name=f"erow{b}")
            nc.scalar.activation(e128[:], sdve[:],
                                 mybir.ActivationFunctionType.Exp,
                                 bias=ngm[:], scale=1.0, accum_out=erow[:])
            st["e128"] = e128
            st["erow"] = erow

        def emit_kw(b):
            st = state[b]
            e_bf = sbsm.tile([P, NKT], bf16, tag="ebf", name=f"ebf{b}")
            nc.scalar.copy(e_bf[:], st["e128"][:])
            kw_ps = [ps_kw.tile([1, A], f32, tag="kw", name=f"kw{b}_{h}")
                     for h in range(2)]
            for t in range(NKT):
                kbf = st["kbf"][t // (NKT // 2)]
                toff = t % (NKT // 2)
                for half in range(2):
                    nc.tensor.matmul(
                        kw_ps[half][:], e_bf[:, t:t + 1],
                        kbf[:, toff * H + half * A: toff * H + (half + 1) * A],
                        start=(t == 0), stop=(t == NKT - 1))
            den_ps = ps_small.tile([1, 1], f32, tag="small")
            nc.tensor.matmul(den_ps[:], st["erow"][:], ones[:],
                             start=True, stop=True)
            den_sb = sbsm.tile([1, 1], f32, tag="den", name=f"den{b}")
            nc.vector.tensor_copy(den_sb[:], den_ps[:])
            st["kw_ps"] = kw_ps
            st["den_sb"] = den_sb

        def emit_fin(b):
            """Final combine for a kw batch; reads kw straight from PSUM."""
            st = state[b]
            kw_ps = st["kw_ps"]
            junk3 = sbjunk.tile([P, H], f32, tag="junk")
            nums = sbsm.tile([1, 2], f32, tag="nums")
            for half in range(2):
                nc.vector.scalar_tensor_tensor(
                    out=junk3[0:1, half * A:(half + 1) * A],
                    in0=kw_ps[half][:], scalar=1.0,
                    in1=u_sb[:, half * A:(half + 1) * A],
                    op0=mybir.AluOpType.mult, op1=mybir.AluOpType.mult,
                    accum_out=nums[:, half:half + 1])
            num = sbsm.tile([1, 1], f32, tag="num")
            nc.vector.tensor_tensor(out=num[:], in0=nums[:, 0:1],
                                    in1=nums[:, 1:2], op=mybir.AluOpType.add)
            _emit_final(num[:], st["den_sb"][:], b)

        def emit_fin3(b):
            """Combine for a classic (v-dot) batch."""
            st = state[b]
            junk5 = sbsm.tile([P, NKT], f32, tag="junk5", bufs=1)
            nrow = sbsm.tile([P, 1], f32, tag="nrow")
            nc.vector.scalar_tensor_tensor(
                out=junk5[:], in0=st["e128"][:], scalar=1.0, in1=st["vdve"][:],
                op0=mybir.AluOpType.mult, op1=mybir.AluOpType.mult,
                accum_out=nrow[:])
            dn_ps = ps_small.tile([1, 2], f32, tag="small")
            nc.tensor.matmul(dn_ps[:, 0:1], st["erow"][:], ones[:],
                             start=True, stop=True)
            nc.tensor.matmul(dn_ps[:, 1:2], nrow[:], ones[:],
                             start=True, stop=True)
            dn = sbsm.tile([1, 2], f32, tag="dn")
            nc.vector.tensor_copy(dn[:], dn_ps[:])
            _emit_final(dn[:, 1:2], dn[:, 0:1], b)

        def _emit_final(num, den, b):
            rden = sbsm.tile([1, 1], f32, tag="rden")
            nc.vector.reciprocal(rden[:], den)
            x = sbsm.tile([1, 1], f32, tag="x")
            nc.vector.tensor_tensor(out=x[:], in0=num, in1=rden[:],
                                    op=mybir.AluOpType.mult)
            x2 = sbsm.tile([1, 1], f32, tag="x2")
            nc.vector.tensor_tensor(out=x2[:], in0=x[:], in1=cv_sb[:],
                                    op=mybir.AluOpType.add)
            nc.sync.dma_start(out=out[b:b + 1, :], in_=x2[:])

        # ---------------- pipeline ----------------
        emit_query(0)

        # Wq natural layout: [h-part, (j a)] ; chunk j at cols [j*A, (j+1)*A)
        Wq_sb = sbc.tile([P, HJ * A], f32)
        nc.sync.dma_start(out=Wq_sb[:].rearrange("p (j a) -> p j a", j=HJ),
                          in_=Wq.rearrange("(j p) a -> p j a", p=P))

        wv_sb = sbc.tile([P, AC], f32)
        nc.sync.dma_start(out=wv_sb[:].rearrange("p (c o) -> p c o", c=AC),
                          in_=Wv.rearrange("(c p) o -> p c o", p=P))
        bk_sb = sbc.tile([P, AC], f32)
        nc.sync.dma_start(out=bk_sb[:], in_=bk.rearrange("(c p) -> p c", p=P))
        bv_sb = sbc.tile([1, 1], f32)
        nc.sync.dma_start(out=bv_sb[:], in_=bv[None, :])
        bq_row = sbc.tile([1, A], f32)
        nc.sync.dma_start(out=bq_row[:], in_=bq[None, :])

        # WkT [a-part, (c h)]: transpose Wk once on the PE (two halves to
        # halve SBUF staging)
        WkT_sb = sbc.tile([P, AC * H], f32)
        if True:
            for half in range(2):
                Wk_sb = sbkey.tile([P, NSUB * H], f32, tag="kg",
                                 name=f"wkstage{half}")
                nc.sync.dma_start(
                    out=Wk_sb[:, 0:(HJ // 2) * A]
                    .rearrange("p (j a) -> p j a", j=HJ // 2),
                    in_=Wk[half * (H // 2):(half + 1) * (H // 2), :]
                    .rearrange("(j p) a -> p j a", p=P))
                for c in range(AC):
                    wkt_ps = ps_small.tile([P, (HJ // 2) * P], f32, tag="small")
                    for jl in range(HJ // 2):
                        nc.tensor.transpose(
                            wkt_ps[:, jl * P:(jl + 1) * P],
                            Wk_sb[:, jl * A + c * P: jl * A + (c + 1) * P],
                            ident[:])
                    dst = WkT_sb[:, c * H + half * (H // 2):
                                 c * H + (half + 1) * (H // 2)]
                    if c % 2 == 0:
                        nc.scalar.copy(dst, wkt_ps[:])
                    else:
                        nc.vector.tensor_copy(dst, wkt_ps[:])

        emit_prep(0)
        emit_query(1)

        # u = Wk @ Wv[:,0]  as a row [1, H]
        u_sb = sbc.tile([1, H], f32)
        for half in range(2):
            u_ps = ps_small.tile([1, A], f32, tag="small")
            for c in range(AC):
                nc.tensor.matmul(
                    u_ps[:], wv_sb[:, c:c + 1],
                    WkT_sb[:, c * H + half * A: c * H + (half + 1) * A],
                    start=(c == 0), stop=(c == AC - 1))
            nc.scalar.copy(u_sb[:, half * A:(half + 1) * A], u_ps[:])

        # cv = bk . Wv + bv
        junk4 = sbsm.tile([P, AC], f32, tag="tiny2")
        cvcol = sbsm.tile([P, 1], f32, tag="tiny3")
        nc.vector.scalar_tensor_tensor(out=junk4[:], in0=bk_sb[:], scalar=1.0,
                                       in1=wv_sb[:], op0=mybir.AluOpType.mult,
                                       op1=mybir.AluOpType.mult, accum_out=cvcol[:])
        cv_ps = ps_small.tile([1, 1], f32, tag="small")
        nc.tensor.matmul(cv_ps[:], cvcol[:], ones[:], start=True, stop=True)
        cv_sb = sbc.tile([1, 1], f32)
        nc.vector.tensor_tensor(out=cv_sb[:], in0=cv_ps[:], in1=bv_sb[:],
                                op=mybir.AluOpType.add)


        wvbc_box[0] = sbw.tile([P, H], f32, tag="wvbc", bufs=1, name="wvbc")
        nc.gpsimd.partition_broadcast(wvbc_box[0][:], u_sb[:], channels=P)


        for b in range(BPC):
            for g in range(NG):
                emit_key_granule(b, g)
                if g == 0:
                    if b + 1 < BPC:
                        emit_prep(b + 1)
                    if b + 2 < BPC:
                        emit_query_granule(b + 2, 0)
                    if b >= 1:
                        emit_softmax(b - 1)
                elif g == 1:
                    if b >= 1:
                        if is_kw(b - 1):
                            emit_kw(b - 1)
                        else:
                            emit_fin3(b - 1)
                    if b + 2 < BPC:
                        emit_query_granule(b + 2, 1)
                elif g == 2:
                    if b >= 2 and is_kw(b - 2):
                        emit_fin(b - 2)
                    if b + 2 < BPC:
                        emit_query_granule(b + 2, 2)
                else:
                    if b + 2 < BPC:
                        emit_query_granule(b + 2, 3)
        if is_kw(BPC - 2):
            emit_fin(BPC - 2)
        emit_softmax(BPC - 1)
        if is_kw(BPC - 1):
            emit_kw(BPC - 1)
            emit_fin(BPC - 1)
        else:
            emit_fin3(BPC - 1)


def _shard(query, key, shared):
    in_maps = []
    for c in range(N_CORES):
        sl = slice(c * BPC, (c + 1) * BPC)
        m = {"query": np.ascontiguousarray(query[sl]),
             "key": np.ascontiguousarray(key[sl])}
        m.update(shared)
        in_maps.append(m)
    return in_maps


def _make_in_maps(inputs):
    query = np.ascontiguousarray(np.asarray(inputs["query"], dtype=np.float32))
    key = np.ascontiguousarray(np.asarray(inputs["key"], dtype=np.float32))
    shared = {k: np.ascontiguousarray(np.asarray(inputs[k], dtype=np.float32))
              for k in ("Wq", "bq", "Wk", "bk", "Wv", "bv")}
    return _shard(query, key, shared)


def kernel(**inputs):
    if "nc" not in _CACHE:
        _CACHE["nc"] = build_bass()
    nc = _CACHE["nc"]
    in_maps = _make_in_maps(inputs)
    res = run_bass_kernel_spmd(nc, in_maps, list(range(N_CORES)))
    outs = [res.results[c]["out"] for c in range(N_CORES)]
    return np.concatenate(outs, axis=0).astype(np.float32)


if __name__ == "__main__":
    rng = np.random.default_rng(0)
    ins = {
        "query": rng.standard_normal((B, LQ, H), dtype=np.float32),
        "key": rng.standard_normal((B, LK, H), dtype=np.float32),
        "Wq": (rng.standard_normal((H, A), dtype=np.float32) / np.sqrt(H)).astype(np.float32),
        "bq": np.zeros((A,), np.float32),
        "Wk": (rng.standard_normal((H, A), dtype=np.float32) / np.sqrt(H)).astype(np.float32),
        "bk": np.zeros((A,), np.float32),
        "Wv": (rng.standard_normal((A, 1), dtype=np.float32) / np.sqrt(A)).astype(np.float32),
        "bv": np.zeros((1,), np.float32),
    }
    x = kernel(**ins)
    print("kernel out:", x[:8, 0])
